# revision 25
# baseline (speedup 1.0000x reference)
"""Chamfer distance kernel for Trainium2 (8 NeuronCores, SPMD).

Problem: B=4 batches, N=M=8192 points, D=3. Per batch:
    d2[n,m] = ||a_n - b_m||^2  (clamped at 0)
    out[b]  = mean_n(min_m d2) + mean_m(min_n d2)

Sharding: core c handles batch c//2, rows [h*4096,(h+1)*4096) of pc1 (h=c%2).
Each core computes, for its 4096x8192 block of the distance matrix:
  - rowacc : per-row min partials (3 per 128-row tile)   -> [128, 96] fp32
  - colacc : per-column min over its 4096 rows (as a
             128-partition-wise partial min)             -> [128, 8192] fp16
Host combines: full col-min = min over partitions and over the 2 cores of a
batch; relu (= the reference's maximum(d2,0), commutes with min) and the
tiny means run on host. colacc's column order is a permutation of m (ACT
region first, then DVE region) - harmless, the final mean is order-invariant.

On-core pipeline per 128-row tile (32 tiles), psum in 2 halves of 4096:
  PE    : 16 matmuls K=13 fp16 hi/lo-split -> psum = a2 - 2 a.b + b2 (fp32).
  ScalarE: escapes the first ACT_COLS/2 columns of each psum half to fp16
          SBUF (activation Copy, 1 elem/cyc @1.2GHz).
  VectorE: one custom DVE op FOLDMIN2 (body out = min(in0,in1) -> colacc
          in-place, accumulator = running MIN over the *in0* lanes only)
          used two ways:
            - ESCFOLD: in0 = psum fp32 remainder columns (1x, 1 elem/cyc):
              folds colacc AND extracts the row-min partial straight from
              PSUM - those columns are never materialized in SBUF.
            - FOLDACC: in0 = escaped fp16 d2 (2x program, 2 elem/cyc):
              folds colacc and extracts the row-min partial in one pass.
          This replaces the old tensor_tensor fold + 7-level row-min tree
          (~8.75k DVE cycles/tile, measured 241.6us/pass) with one 8.25k-
          cycle instruction (measured ~233us/pass pooled over 72 slope
          rounds; +~22us fixed overhead).

Measured per-engine ceilings (no_dve / no_act timing builds): ScalarE
6.2us/tile, VectorE 7.7us/tile, full kernel 7.0-7.4us/tile -> DVE-bound.
This is the architectural floor for the structure: both reductions must
each touch every d2 element on the DVE (no two-tensor min exists on any
other engine; TensorTensor does not lower on ScalarE; GpSimd's two-input
floor is ~2.5 cyc per 16-wide group and it shares the DVE SBUF port).
The one remaining ~1.9x lever is a 2X_1PORT custom op with a working
accumulator (fold + row-min at 2 elem/cyc), but the 2x accumulator
readout is broken in the current firmware: a/out-flops are cleared
between instructions, READ_ACCUMULATOR2 returns packed-fp16 junk after
a 2x op, and swap-flop accumulators see scrambled chain data at slices
>=2 in 2x mode (probe scripts under work/).
"""

import numpy as np

B, N, M, D = 4, 8192, 8192, 3
NCORES = 8
NH = N // 2          # rows per core
NT = NH // 128       # 32 n-tiles of 128 rows
K = 13               # split-matmul contraction size (a2 folded in)
ACT_COLS = 8192      # columns escaped via ScalarE (rest folded from PSUM)

_CACHE = {}


def _register_foldmin_op():
    """Register the FOLDMIN2_ANT custom DVE op:
        out[j]    = min(in0[j], in1[j])          (colacc fold, in-place ok)
        accum_out = min(s1, min_j in0[j])        (row-min of the d2 stream)
    The accumulator taps ONLY the in0 (d2) lanes - in1 (colacc) carries
    mins of *other* rows, which must not leak into the row-min.
    Hand-written 1x program (fp32 psum in0) and 2X_1PORT program (fp16).
    Idempotent per process."""
    import concourse.dve_ops as dve_ops_mod
    from concourse.dve_spec import Spec, Src0, Src1, C1, minn
    from concourse.dve_spec import AluOp as SAluOp
    from concourse.dve_uop import (
        DveOpSpec, UopConfig, UopDpConfig, InpSel, OutPath, OutSel,
        AluInp, DelayInp, Trigger,
    )

    NAME = "FOLDMIN2_ANT"
    if NAME in dve_ops_mod._SUB_OPCODE_FOR_NAME:
        for op in dve_ops_mod.OPS:
            if op.name == NAME:
                return op

    def _ref(in0, in1, s0, s1, imm2):
        b = np.minimum(in0.astype(np.float32), in1.astype(np.float32))
        acc = np.minimum(
            np.float32(s1),
            in0.astype(np.float32).reshape(in0.shape[0], -1)
            .min(axis=-1, keepdims=True),
        ).astype(np.float32)
        return b, acc

    spec = Spec(body=minn(Src0, Src1), accum=SAluOp.MIN, accum_init=C1,
                reference=_ref)

    MIN, BYP = SAluOp.MIN, SAluOp.BYPASS
    PREV = AluInp.PREV_ALU_OUT

    def dp_relay(*keep):
        d = UopDpConfig()
        d.enable_alu(BYP, PREV, PREV)
        d.alu_out_a_enable = 1
        if keep:
            d.pass_through_delay(*keep)
        return d

    def _build_1x():
        """1x program: chains 0=SRC_0(d2/psum) 1=SRC_1(colacc) 2=CONST_1.
        s0: out=min(S0,S1); s1: acc=min(acc,S0) [CURR_ALU_OUT flop],
        capture out into chain3; s2-7 relay acc, pass chain3 to WR0_LO."""
        st = UopConfig()
        st.enable_input(InpSel.SRC_0, 1).enable_input(InpSel.SRC_1, 2)
        st.enable_input(InpSel.CONST_1, 3)
        s = st.datapath_config
        s[0] = (UopDpConfig()
                .enable_alu(MIN, AluInp.PREV_DELAY_0, AluInp.PREV_DELAY_1)
                .pass_through_delay(0))
        s[1] = (UopDpConfig()
                .enable_alu(MIN, AluInp.CURR_ALU_OUT, AluInp.PREV_DELAY_0)
                .enable_delay_from_src(DelayInp.PREV_ALU_OUT, 3))
        s[1].alu_out_a_enable = 1
        for i in range(2, 8):
            s[i] = dp_relay(3)
        st.enable_output(OutSel.DELAY_3, OutPath.WR0_LO)
        st.require_inp0 = 1
        st.require_inp1 = 1
        st.trigger = (Trigger.SRC_TENSOR_DONE, Trigger.NONE, Trigger.NONE)
        st.next_uop = (0, 0, 0)
        st.accum_enabled = 1
        st.repeat_count = 0

        # seed: acc flop (stage 1) <- CONST_1
        sd = UopConfig()
        sd.enable_input(InpSel.SRC_0, 1).enable_input(InpSel.SRC_1, 2)
        sd.enable_input(InpSel.CONST_1, 3)
        d = sd.datapath_config
        d[0] = (UopDpConfig()
                .enable_alu(BYP, PREV, PREV)
                .pass_through_delay(2))
        d[1] = UopDpConfig().enable_alu(BYP, AluInp.PREV_DELAY_2,
                                        AluInp.PREV_DELAY_2)
        d[1].alu_out_a_enable = 1
        for i in range(2, 8):
            d[i] = dp_relay()
        sd.require_inp0 = 0
        sd.require_inp1 = 0
        sd.trigger = (Trigger.COUNT, Trigger.NONE, Trigger.NONE)
        sd.repeat_count = 1
        sd.next_uop = (1, 0, 0)
        sd.accum_enabled = 1
        return [sd, st]

    def _build_2x():
        """2X_1PORT program: 2 fp16 pairs per cycle.
        chains: 0=SRC_0(d2 lo) 1=SRC_1(cacc lo) 2=SRC_0_HI(d2 hi)
                3=SRC_1_HI(cacc hi) 4=CONST_1.
        s0: lo=min(S0,S1); s1: hi=min(S0H,S1H), chain5<-lo;
        s2: pair=min(S0,S0H) [d2 lanes only], chain1<-hi;
        s3: acc=min(acc,pair); s4-7 relay. WR0_LO<-ch5, WR0_HI<-ch1."""
        import os
        variant = os.environ.get("KERNEL_FOLDMIN_V2X", "pd")
        st = UopConfig()
        st.enable_input(InpSel.SRC_0, 1).enable_input(InpSel.SRC_1, 2)
        st.enable_input(InpSel.SRC_0_HI, 3).enable_input(InpSel.SRC_1_HI, 4)
        st.enable_input(InpSel.CONST_1, 5)
        s = st.datapath_config
        s[0] = (UopDpConfig()
                .enable_alu(MIN, AluInp.PREV_DELAY_0, AluInp.PREV_DELAY_1)
                .pass_through_delay(0, 2, 3))
        s[1] = (UopDpConfig()
                .enable_alu(MIN, AluInp.PREV_DELAY_2, AluInp.PREV_DELAY_3)
                .enable_delay_from_src(DelayInp.PREV_ALU_OUT, 5)
                .pass_through_delay(0, 2))
        if variant == "lo":      # debug: accum over lo lanes only
            pair_a, pair_b = AluInp.PREV_DELAY_0, AluInp.PREV_DELAY_0
        elif variant == "body":  # debug: accum over body outs (contaminated)
            pair_a, pair_b = PREV, AluInp.PREV_DELAY_5
        else:                    # pd / seed8: accum over the d2 (Src0) lanes
            pair_a, pair_b = AluInp.PREV_DELAY_0, AluInp.PREV_DELAY_2
        s[2] = (UopDpConfig()
                .enable_alu(MIN, pair_a, pair_b)
                .enable_delay_from_src(DelayInp.PREV_ALU_OUT, 1)
                .pass_through_delay(5))
        s[3] = (UopDpConfig()
                .enable_alu(MIN, AluInp.CURR_ALU_OUT, PREV)
                .pass_through_delay(1, 5))
        s[3].alu_out_a_enable = 1
        for i in range(4, 8):
            s[i] = dp_relay(1, 5)
        st.enable_output(OutSel.DELAY_5, OutPath.WR0_LO)
        st.enable_output(OutSel.DELAY_1, OutPath.WR0_HI)
        st.require_inp0 = 1
        st.require_inp1 = 1
        st.trigger = (Trigger.SRC_TENSOR_DONE, Trigger.NONE, Trigger.NONE)
        st.next_uop = (0, 0, 0)
        st.accum_enabled = 1
        st.repeat_count = 0

        # seed: acc flop (stage 3) <- CONST_1 (rides chain 4)
        sd = UopConfig()
        sd.enable_input(InpSel.SRC_0, 1).enable_input(InpSel.SRC_1, 2)
        sd.enable_input(InpSel.SRC_0_HI, 3).enable_input(InpSel.SRC_1_HI, 4)
        sd.enable_input(InpSel.CONST_1, 5)
        d = sd.datapath_config
        for i in range(3):
            d[i] = (UopDpConfig()
                    .enable_alu(BYP, PREV, PREV)
                    .pass_through_delay(4))
        d[3] = UopDpConfig().enable_alu(BYP, AluInp.PREV_DELAY_4,
                                        AluInp.PREV_DELAY_4)
        d[3].alu_out_a_enable = 1
        for i in range(4, 8):
            d[i] = dp_relay()
        sd.require_inp0 = 0
        sd.require_inp1 = 0
        sd.trigger = (Trigger.COUNT, Trigger.NONE, Trigger.NONE)
        sd.repeat_count = 8 if variant == "seed8" else 1
        sd.next_uop = (1, 0, 0)
        sd.accum_enabled = 1
        return [sd, st]

    row = max(dve_ops_mod._SUB_OPCODE_FOR_NAME.values()) + 1
    assert row < 0x20

    class _FoldMinOp:
        name = NAME
        subdim = False

        def __init__(self):
            self.spec = spec
            self._cache = {}

        def compile(self, ver):
            if ver in self._cache:
                return self._cache[ver]
            assert ver == "v3", "FOLDMIN2_ANT only has v3 (TRN2) programs"
            r = DveOpSpec(
                name=NAME,
                opcode=row,
                uops=_build_1x(),
                rd1_en=True,
                uops_2x=_build_2x(),
                perf_max=1,
            )
            self._cache[ver] = r
            return r

    op = _FoldMinOp()
    dve_ops_mod.OPS.append(op)
    dve_ops_mod.CUSTOM_DVE_SPECS[NAME] = spec
    dve_ops_mod._SUB_OPCODE_FOR_NAME[NAME] = row
    return op


def _build(reps=1, tiny_out=False, act_cols=ACT_COLS, alloc_mode="stack",
           no_dve=False, no_act=False):
    """Build + compile the SPMD NEFF once per process.

    reps>1 repeats the main loop (identical results) -- used only for
    slope-based execution timing; the product path uses reps=1."""
    import concourse.bacc as bacc
    import concourse.tile as tile
    import concourse.mybir as mybir

    foldmin = _register_foldmin_op()

    nc = bacc.Bacc("TRN2", target_bir_lowering=False, debug=False,
                   num_devices=NCORES)
    f16, f32 = mybir.dt.float16, mybir.dt.float32

    # Per psum chunk c (4 chunks of 2048 cols): the first act_cols columns
    # of the tile (in chunk order) escape via ScalarE, the rest fold from
    # PSUM via the 1x custom op.
    CH = 2048
    acts = [max(0, min(act_cols - c * CH, CH)) for c in range(4)]
    dves = [CH - a for a in acts]
    dve_off = [act_cols + sum(dves[:c]) for c in range(4)]
    NACC = sum(1 for d in dves if d) + (1 if act_cols else 0)
    # First/last tiles run chunk-split FOLDACCs (4 accum partials each):
    # tile 0's fold starts after chunk 0's escape instead of chunk 3's
    # (shorter pipeline fill), tile NT-1's colacc DMA slices fire as each
    # chunk's fold completes (tail overlap). Only wired for the pure-ACT
    # config (no ESCFOLD columns).
    split_tiles = {0, NT - 1} if act_cols == 4 * CH and not no_dve else set()
    NROWACC = NACC * NT + 3 * len(split_tiles)

    def racc_cols(i):
        """rowacc columns holding tile i's row-min partials."""
        lo = NACC * i + 3 * sum(1 for t in split_tiles if t < i)
        return lo, lo + (3 + NACC if i in split_tiles else NACC)

    w_d = nc.dram_tensor("w", [K, NH], f16, kind="ExternalInput")
    bh_d = nc.dram_tensor("bh", [K, M], f16, kind="ExternalInput")
    colacc_shape = [128, 32] if tiny_out else [128, M]
    colacc_d = nc.dram_tensor("colacc", colacc_shape, f16,
                              kind="ExternalOutput")
    rowacc_d = nc.dram_tensor("rowacc", [128, NROWACC], f32,
                              kind="ExternalOutput")

    BIG = 3.0e38
    BIG16 = 60000.0

    with tile.TileContext(nc, pool_alloc_mode=alloc_mode) as tc:
        with (
            tc.tile_pool(name="consts", bufs=1) as consts,
            tc.tile_pool(name="psum", bufs=2, space="PSUM") as psum_pool,
            tc.tile_pool(name="d2", bufs=2) as d2_pool,
        ):
            w_sb = consts.tile([K, NH], f16)
            nc.sync.dma_start(out=w_sb, in_=w_d.ap())
            bh_sb = consts.tile([K, M], f16)
            nc.sync.dma_start(out=bh_sb, in_=bh_d.ap())

            colacc = consts.tile([128, M], f16)
            rowacc = consts.tile([128, NROWACC], f32)
            # one-time init: fold identity (overlaps the input DMAs)
            nc.vector.memset(colacc, BIG16)
            if no_dve:
                nc.vector.memset(rowacc, 0.0)

            n_iters = reps * NT
            it = 0
            for i in [t for _ in range(reps) for t in range(NT)]:
                is_last_iter = (it == n_iters - 1)
                it += 1
                d2row = None
                if act_cols:
                    d2row = d2_pool.tile([128, act_cols], f16, tag="d2row",
                                         name="d2row")
                lo, _hi = racc_cols(i)
                nacc = 0
                for c in range(4):
                    ps = psum_pool.tile([128, CH], f32, tag="ps")
                    for j in range(4):
                        q = c * 4 + j
                        nc.tensor.matmul(
                            ps[:, j * 512:(j + 1) * 512],
                            w_sb[:, i * 128:(i + 1) * 128],
                            bh_sb[:, q * 512:(q + 1) * 512],
                            start=True, stop=True,
                        )
                    if acts[c] and (not no_act or c == 0):
                        aw = acts[c] if not no_act else 64
                        nc.scalar.activation(
                            out=d2row[:, c * CH:c * CH + aw],
                            in_=ps[:, :aw],
                            func=mybir.ActivationFunctionType.Copy,
                            bias=0.0, scale=1.0,
                        )
                    if dves[c] and not no_dve:
                        off = dve_off[c]
                        col = lo + nacc
                        nacc += 1
                        nc.vector._custom_dve(
                            foldmin,
                            out=colacc[:, off:off + dves[c]],
                            in0=ps[:, acts[c]:CH],
                            in1=colacc[:, off:off + dves[c]],
                            s1=BIG,
                            accum_out=rowacc[:, col:col + 1],
                        )
                    if i in split_tiles and not no_act:
                        # chunk-split fold: tile 0 shortens pipeline fill,
                        # tile NT-1 lets each colacc DMA slice overlap the
                        # remaining chunk folds (tail overlap)
                        col = lo + nacc
                        nacc += 1
                        nc.vector._custom_dve(
                            foldmin,
                            out=colacc[:, c * CH:(c + 1) * CH],
                            in0=d2row[:, c * CH:(c + 1) * CH],
                            in1=colacc[:, c * CH:(c + 1) * CH],
                            s1=BIG,
                            accum_out=rowacc[:, col:col + 1],
                        )
                        if is_last_iter and not tiny_out:
                            nc.sync.dma_start(
                                out=colacc_d.ap()[:, c * CH:(c + 1) * CH],
                                in_=colacc[:, c * CH:(c + 1) * CH])
                if act_cols and not no_dve and (i not in split_tiles
                                                or no_act):
                    # 1x custom op: fold + row-min in one pass. (The 2x
                    # program exists but the 2x accumulator readout is
                    # broken on HW - perf_max stays 0, forcing 1x.)
                    col = lo + nacc
                    nc.vector._custom_dve(
                        foldmin,
                        out=colacc[:, :act_cols],
                        in0=d2row,
                        in1=colacc[:, :act_cols],
                        s1=BIG,
                        accum_out=rowacc[:, col:col + 1],
                    )

            if tiny_out:
                nc.sync.dma_start(out=colacc_d.ap(), in_=colacc[:, :32])
            elif (NT - 1) not in split_tiles or no_act:
                # split across DMA queues so the 2MB tail drains in parallel
                q = M // 4
                for c in range(4):
                    nc.sync.dma_start(out=colacc_d.ap()[:, c * q:(c + 1) * q],
                                      in_=colacc[:, c * q:(c + 1) * q])
            nc.sync.dma_start(out=rowacc_d.ap(), in_=rowacc)

    nc.compile()
    return nc


def _prep_inputs(pc1, pc2):
    """Host-side: build per-core fp16 hi/lo split operands (tiny arrays)."""
    in_maps = []
    for c in range(NCORES):
        b, h = divmod(c, 2)
        a = np.asarray(pc1[b][h * NH:(h + 1) * NH], dtype=np.float32)  # [NH,3]
        bb = np.asarray(pc2[b], dtype=np.float32)                      # [M,3]

        ah = a.astype(np.float16)
        al = (a - ah.astype(np.float32)).astype(np.float16)
        a2 = np.square(a.astype(np.float64)).sum(-1)                   # [NH]
        a2h = a2.astype(np.float16)
        a2l = (a2 - a2h.astype(np.float64)).astype(np.float16)
        w = np.empty((K, NH), dtype=np.float16)
        w[0:3] = (ah.T * np.float16(-2))
        w[3:6] = (al.T * np.float16(-2))
        w[6:9] = (ah.T * np.float16(-2))
        w[9] = np.float16(1.0)
        w[10] = np.float16(1.0)
        w[11] = a2h
        w[12] = a2l

        bhh = bb.astype(np.float16)
        bl = (bb - bhh.astype(np.float32)).astype(np.float16)
        b2 = np.square(bb.astype(np.float64)).sum(-1)                  # [M]
        b2h = b2.astype(np.float16)
        b2l = (b2 - b2h.astype(np.float64)).astype(np.float16)
        bh = np.empty((K, M), dtype=np.float16)
        bh[0:3] = bhh.T
        bh[3:6] = bhh.T
        bh[6:9] = bl.T
        bh[9] = b2h
        bh[10] = b2l
        bh[11] = np.float16(1.0)
        bh[12] = np.float16(1.0)

        in_maps.append({"w": w, "bh": bh})
    return in_maps


def _run(in_maps, trace=False):
    from concourse.bass_utils import run_bass_kernel_spmd
    if "nc" not in _CACHE:
        _CACHE["nc"] = _build()
    return run_bass_kernel_spmd(_CACHE["nc"], in_maps,
                                core_ids=list(range(NCORES)), trace=trace)


def kernel(pc1, pc2, _trace=False):
    pc1 = np.asarray(pc1, dtype=np.float32)
    pc2 = np.asarray(pc2, dtype=np.float32)
    res = _run(_prep_inputs(pc1, pc2), trace=_trace)

    out = np.empty((B,), dtype=np.float32)
    for b in range(B):
        r0, r1 = res.results[2 * b], res.results[2 * b + 1]
        colmin = np.minimum(
            r0["colacc"].astype(np.float32).min(axis=0),
            r1["colacc"].astype(np.float32).min(axis=0),
        )                                                              # [M]
        term2 = np.maximum(colmin, 0.0).mean(dtype=np.float64)
        # row-min: min over the NACC per-tile partials, rows r = i*128+p
        rmins = []
        for r in (r0, r1):
            ra = r["rowacc"]
            if ra.shape[1] % NT == 0:                     # uniform partials
                nacc = ra.shape[1] // NT
                rmins.append(ra.reshape(128, NT, nacc).min(axis=2))
            else:           # chunk-split first/last tiles (4 partials each)
                rm = np.empty((128, NT), dtype=ra.dtype)
                for i in range(NT):
                    lo = i + (3 if i > 0 else 0)
                    n = 4 if i in (0, NT - 1) else 1
                    rm[:, i] = ra[:, lo:lo + n].min(axis=1)
                rmins.append(rm)
        rowmins = np.concatenate([rm.T.ravel() for rm in rmins])       # [N]
        term1 = np.maximum(rowmins, 0.0).mean(dtype=np.float64)
        out[b] = np.float32(term1 + term2)
    kernel._last_results = res
    return out
